# revision 1
# baseline (speedup 1.0000x reference)
"""Correlation layer + softmax(axis=i) Trainium2 kernel.

corr[b,i,j] = sum_c f1[b,c,i] * f2[b,c,j]   (b=4, c=256, i,j in hw=4096)
out = softmax(corr, axis=i) reshaped to (4, 4096, 64, 64)

Sharding: 8 cores = 4 batches x 2 j-halves. Softmax reduces over i, which is
fully local per core when corr is computed transposed (j on partitions, i on
the free axis). Per core, for each of 16 j-tiles (128 j's):
  1. corrT tile (128 j, 4096 i) = f2_tile.T @ f1 via 16 matmuls (fp32r,
     1 cyc/row) into 8 resident PSUM banks,
  2. per-column max via reduce_max straight off PSUM (negated -> exp bias),
  3. exp(corr - max) PSUM -> SBUF on ACT, accumulating row sums on the fly,
  4. 1/sum via an ACT-only chain (recip = exp(-ln(total))), normalize in
     place on the Pool engine, DMA the (128 j, 4096 i) tile out contiguous.
The device output is (2048 j, 4096 i) per core; the host transposes during
unsharding (the gather has to copy these bytes anyway).

This walrus build allows only ONE sync wait per instruction. Tile freely
emits several, so kernel.py patches two spots in the Tile pipeline:
  - a post-scheduling pass splits every multi-wait instruction into
    single-wait same-engine Drain carriers ahead of it,
  - the kernel-tail drain (one wait per outstanding semaphore) is split the
    same way.
"""

import sys

import numpy as np

sys.path.insert(0, "/opt/trn_rl_repo")

import concourse.bass as bass
import concourse.mybir as mybir
import concourse.tile as tile
from concourse.bass_utils import run_bass_kernel_spmd

B, C, H, W = 4, 256, 64, 64
HW = H * W  # 4096
JJ = HW // 2  # j columns per core
N_CORES = 8
P = 128
KC = C // P  # 2 contraction chunks
NJT = JJ // P  # 16 j-tiles per core
IC = 512  # i-chunk width (one PSUM bank)
NIC = HW // IC  # 8 i-chunks
MMN = 512  # matmul moving width (one PSUM bank)
USE_FP32R = True  # fp32r matmul: 1 cyc/row vs 4 for fp32

FP32 = mybir.dt.float32

_split_counter = [0]


def _split_multiwaits(ordered):
    """Walrus (this build) rejects instructions with >1 sync wait. Hoist the
    extra waits onto single-wait Drain instructions on the same engine queue
    immediately before the offender (queues are in-order)."""
    for bb, insts in ordered.items():
        out = []
        changed = False
        for inst in insts:
            si = getattr(inst, "sync_info", None)
            waits = list(si.on_wait) if (si is not None and si.on_wait) else []
            if len(waits) > 1:
                changed = True
                for w in waits[:-1]:
                    _split_counter[0] += 1
                    d = mybir.InstDrain(
                        name=f"I-wsplit-{_split_counter[0]}",
                        ins=[],
                        outs=[],
                        engine=inst.engine,
                    )
                    d.sync_info = mybir.SyncInfo(on_wait=[w], on_update=[])
                    out.append(d)
                si.on_wait = waits[-1:]
            out.append(inst)
        if changed:
            ordered[bb] = out
    return ordered


_orig_postorder = tile.postorder_instruction_blocks


def _patched_postorder(ordered, start_bb_name, postordered):
    _split_multiwaits(ordered)
    return _orig_postorder(ordered, start_bb_name, postordered)


tile.postorder_instruction_blocks = _patched_postorder


def _patched_drain_and_barrier(self, tick_clock, wait_clock):
    """Same single-wait discipline for the kernel-tail drain."""
    from concourse.vector_clock import ScopedClock

    drain_inst = self.nc.sync.drain()
    wait_clock.add_sem_waits(
        drain_inst.ins, ScopedClock({None: tick_clock.global_clock})
    )
    si = drain_inst.ins.sync_info
    waits = list(si.on_wait or []) if si is not None else []
    if len(waits) > 1:
        si.on_wait = waits[:1]
        for w in waits[1:]:
            d2 = self.nc.sync.drain()
            si2 = d2.ins.sync_info
            if si2 is None:
                d2.ins.sync_info = mybir.SyncInfo(on_wait=[w], on_update=[])
            else:
                si2.on_wait = [w]
    self.nc.all_engine_barrier()
    assert self.sems is not None
    popped = self.nc._tile_sem_poison_stack.pop()
    assert popped is self._sem_poison
    self.nc.clear_and_free_semaphores(list(self.sems.allocated().values()))
    self.nc.all_engine_barrier()


tile.TileContext._drain_and_barrier = _patched_drain_and_barrier


def _build_bass():
    nc = bass.Bass()
    mmdt = mybir.dt.float32r if USE_FP32R else FP32
    fin = nc.declare_dram_parameter("fin", [C, HW + JJ], mmdt, isOutput=False)
    out = nc.declare_dram_parameter("out", [JJ, HW], FP32, isOutput=True)

    with tile.TileContext(nc) as tc:
        with (
            tc.tile_pool(name="singles", bufs=1) as singles,
            tc.tile_pool(name="exp", bufs=3) as exp_pool,
            tc.tile_pool(name="stats", bufs=16) as stats,
            tc.tile_pool(name="ps", bufs=8, space="PSUM") as ps_pool,
        ):
            # Preload f1|f2, one SBUF tile per 128-row contraction chunk.
            # SWDGE (gpsimd) keeps the SP queue free for the output stream.
            fin_sb = []
            for cc in range(KC):
                t = singles.tile([P, HW + JJ], mmdt, tag=f"fin_{cc}")
                eng = nc.gpsimd if cc == 0 else nc.scalar
                eng.dma_start(out=t, in_=fin[cc * P : (cc + 1) * P, :])
                fin_sb.append(t)

            for jt in range(NJT):
                # 1. corrT j-tile into 8 resident PSUM banks
                ps_list = []
                for ic in range(NIC):
                    ps = ps_pool.tile([P, IC], FP32, tag="ps")
                    for sub in range(IC // MMN):
                        for cc in range(KC):
                            nc.tensor.matmul(
                                ps[:, bass.ts(sub, MMN)],
                                lhsT=fin_sb[cc][:, HW + jt * P : HW + (jt + 1) * P],
                                rhs=fin_sb[cc][
                                    :, ic * IC + sub * MMN : ic * IC + (sub + 1) * MMN
                                ],
                                start=(cc == 0),
                                stop=(cc == KC - 1),
                            )
                    ps_list.append(ps)
                # 2. per-column (per-partition here) max, straight off PSUM
                mx = stats.tile([P, NIC], FP32)
                for ic in range(NIC):
                    nc.vector.reduce_max(
                        out=mx[:, ic : ic + 1],
                        in_=ps_list[ic],
                        axis=mybir.AxisListType.X,
                    )
                negmax = stats.tile([P, 1], FP32)
                nc.vector.reduce_max(
                    out=negmax, in_=mx, axis=mybir.AxisListType.X, negate=True
                )
                # 3. exp(corr - max) PSUM -> SBUF, accumulating row sums
                exp_t = exp_pool.tile([P, HW], FP32)
                sums = stats.tile([P, NIC], FP32)
                for ic in range(NIC):
                    nc.scalar.activation(
                        out=exp_t[:, bass.ts(ic, IC)],
                        in_=ps_list[ic],
                        func=mybir.ActivationFunctionType.Exp,
                        bias=negmax,
                        scale=1.0,
                        accum_out=sums[:, ic : ic + 1],
                    )
                # 4. recip = exp(-ln(total)) via ACT-only chain, normalize in
                # place on Pool, stream out over the HWDGE queues.
                total = stats.tile([P, 1], FP32)
                nc.vector.reduce_sum(out=total, in_=sums, axis=mybir.AxisListType.X)
                lntot = stats.tile([P, 1], FP32)
                nc.scalar.activation(
                    out=lntot, in_=total, func=mybir.ActivationFunctionType.Ln
                )
                recip = stats.tile([P, 1], FP32)
                nc.scalar.activation(
                    out=recip,
                    in_=lntot,
                    func=mybir.ActivationFunctionType.Exp,
                    scale=-1.0,
                )
                nc.gpsimd.tensor_scalar_mul(out=exp_t, in0=exp_t, scalar1=recip)
                # 5. fully contiguous DMA out (row j = jt*128 + p)
                nc.sync.dma_start(
                    out=out[jt * P : (jt + 1) * P, :],
                    in_=exp_t,
                )
    return nc


_NC = None


def _get_nc():
    global _NC
    if _NC is None:
        _NC = _build_bass()
    return _NC


def _run(feat1, feat2, trace=False):
    f1 = np.asarray(feat1, dtype=np.float32).reshape(B, C, HW)
    f2 = np.asarray(feat2, dtype=np.float32).reshape(B, C, HW)
    in_maps = []
    for d in range(N_CORES):
        bb, jh = d // 2, d % 2
        fin = np.concatenate([f1[bb], f2[bb][:, jh * JJ : (jh + 1) * JJ]], axis=1)
        in_maps.append({"fin": np.ascontiguousarray(fin)})
    res = run_bass_kernel_spmd(_get_nc(), in_maps, list(range(N_CORES)), trace=trace)
    out = np.empty((B, HW, HW), np.float32)
    for d in range(N_CORES):
        bb, jh = d // 2, d % 2
        # device tile is (j_local, i); transpose during unshard
        out[bb][:, jh * JJ : (jh + 1) * JJ] = res.results[d]["out"].T
    return out.reshape(B, HW, H, W), res


def kernel(feat1, feat2):
    out, _ = _run(feat1, feat2)
    return out



# revision 12
# speedup vs baseline: 1.2253x; 1.2253x over previous
"""Correlation layer + softmax(axis=i) Trainium2 kernel.

corr[b,i,j] = sum_c f1[b,c,i] * f2[b,c,j]   (b=4, c=256, i,j in hw=4096)
out = softmax(corr, axis=i) reshaped to (4, 4096, 64, 64)

Sharding: 8 cores = 4 batches x 2 j-halves. Softmax reduces over i, which is
fully local per core when corr is computed transposed (j on partitions, i on
the free axis).

Per core (2048 j x 4096 i), for each of 16 j-tiles (128 j), a chunked
(flash-style) softmax over four 1024-wide i-quarters:
  1. corrT quarter (128 j, 1024 i) = f2_cols.T @ f1 via 4 fp32r matmuls into
     a 2-bank PSUM tile (pool bufs=4 -> all 8 banks, quarters pipeline
     independently),
  2. per-quarter row max m_q (DVE, negated) then exp(corr - m_q) straight off
     PSUM in ONE activation per quarter (amortizes PSUM-access + accumulator
     overhead), per-quarter row sums accumulate on the fly. Using the LOCAL
     quarter max keeps every exp in [0,1] -- overflow-safe for any input and,
     unlike a global row max, never serializes PSUM recycling across quarters.
  3. merge: M = max_q m_q, e_q = exp(m_q - M), S = sum_q sums_q*e_q,
     r_q = e_q / S  (tiny [128,4] ops on DVE + one small ACT exp),
  4. normalize quarter q by r_q -- Pool takes q0..q2, DVE takes q3 (balances
     both engines under the DMA roofline) -- and DMA each quarter out as soon
     as it is scaled.
Input DMAs are chunked in need-order (j-tile-0 cols, f1 halves, remaining f2
cols) so matmuls start ~2us in; output DMA saturates from ~20us on. The
device output is (2048 j, 4096 i) per core; the host transposes during
unsharding (the gather has to copy these bytes anyway).

This walrus build allows only ONE sync wait per instruction. Tile freely
emits several, so kernel.py patches two spots in the Tile pipeline:
  - a post-scheduling pass splits every multi-wait instruction into
    single-wait same-engine Drain carriers ahead of it,
  - the kernel-tail drain (one wait per outstanding semaphore) is split the
    same way.
"""

import sys

import numpy as np

sys.path.insert(0, "/opt/trn_rl_repo")

import concourse.bass as bass
import concourse.mybir as mybir
import concourse.tile as tile
from concourse.bass_utils import run_bass_kernel_spmd

B, C, H, W = 4, 256, 64, 64
HW = H * W  # 4096
JJ = HW // 2  # j columns per core
N_CORES = 8
P = 128
KC = C // P  # 2 contraction chunks
NJT = JJ // P  # 16 j-tiles per core
HB = 2048  # input-load half width
QW = 1024  # softmax quarter width = 2 PSUM banks
NQ = HW // QW  # 4
MMN = 512  # matmul moving width (one PSUM bank)

FP32 = mybir.dt.float32

_split_counter = [0]


def _split_multiwaits(ordered):
    """Walrus (this build) rejects instructions with >1 sync wait. Hoist the
    extra waits onto single-wait Drain instructions on the same engine queue
    immediately before the offender (queues are in-order)."""
    for bb, insts in ordered.items():
        out = []
        changed = False
        for inst in insts:
            si = getattr(inst, "sync_info", None)
            waits = list(si.on_wait) if (si is not None and si.on_wait) else []
            if len(waits) > 1:
                changed = True
                for w in waits[:-1]:
                    _split_counter[0] += 1
                    d = mybir.InstDrain(
                        name=f"I-wsplit-{_split_counter[0]}",
                        ins=[],
                        outs=[],
                        engine=inst.engine,
                    )
                    d.sync_info = mybir.SyncInfo(on_wait=[w], on_update=[])
                    out.append(d)
                si.on_wait = waits[-1:]
            out.append(inst)
        if changed:
            ordered[bb] = out
    return ordered


_orig_postorder = tile.postorder_instruction_blocks


def _patched_postorder(ordered, start_bb_name, postordered):
    _split_multiwaits(ordered)
    return _orig_postorder(ordered, start_bb_name, postordered)


tile.postorder_instruction_blocks = _patched_postorder


def _patched_drain_and_barrier(self, tick_clock, wait_clock):
    """Same single-wait discipline for the kernel-tail drain."""
    from concourse.vector_clock import ScopedClock

    drain_inst = self.nc.sync.drain()
    wait_clock.add_sem_waits(
        drain_inst.ins, ScopedClock({None: tick_clock.global_clock})
    )
    si = drain_inst.ins.sync_info
    waits = list(si.on_wait or []) if si is not None else []
    if len(waits) > 1:
        si.on_wait = waits[:1]
        for w in waits[1:]:
            d2 = self.nc.sync.drain()
            si2 = d2.ins.sync_info
            if si2 is None:
                d2.ins.sync_info = mybir.SyncInfo(on_wait=[w], on_update=[])
            else:
                si2.on_wait = [w]
    self.nc.all_engine_barrier()
    assert self.sems is not None
    popped = self.nc._tile_sem_poison_stack.pop()
    assert popped is self._sem_poison
    self.nc.clear_and_free_semaphores(list(self.sems.allocated().values()))
    self.nc.all_engine_barrier()


tile.TileContext._drain_and_barrier = _patched_drain_and_barrier


def _build_bass():
    nc = bass.Bass()
    mmdt = mybir.dt.float32r
    fj = nc.declare_dram_parameter("fj", [C, JJ], mmdt, isOutput=False)
    fi = nc.declare_dram_parameter("fi", [C, HW], mmdt, isOutput=False)
    out = nc.declare_dram_parameter("out", [JJ, HW], FP32, isOutput=True)

    with tile.TileContext(nc) as tc:
        with (
            tc.tile_pool(name="singles", bufs=1) as singles,
            tc.tile_pool(name="exp", bufs=4) as exp_pool,
            tc.tile_pool(name="stats", bufs=24) as stats,
            tc.tile_pool(name="ps", bufs=4, space="PSUM") as ps_pool,
        ):
            # Input DMAs in need-order. Separate tiles per chunk so a matmul
            # only waits on the chunk it actually reads.
            fj0_sb, fja_sb, fjb_sb, fi_sb = [], [], [], []
            # fj0 on the SWDGE (gpsimd) queue so it issues in parallel with
            # the SP queue's fi stream.
            for cc in range(KC):
                t = singles.tile([P, P], mmdt, tag=f"fj0_{cc}")
                nc.gpsimd.dma_start(out=t, in_=fj[cc * P : (cc + 1) * P, 0:P])
                fj0_sb.append(t)
            for half in range(2):
                for cc in range(KC):
                    t = singles.tile([P, HB], mmdt, tag=f"fi_{half}_{cc}")
                    nc.sync.dma_start(
                        out=t,
                        in_=fi[cc * P : (cc + 1) * P, half * HB : (half + 1) * HB],
                    )
                    fi_sb.append(t)  # index = half*KC + cc
            # all of fi lands first; the fj tail then streams in during tile
            # 0's softmax-merge latency instead of in front of it.
            FJA = 4 * P
            for cc in range(KC):
                t = singles.tile([P, FJA - P], mmdt, tag=f"fja_{cc}")
                nc.sync.dma_start(out=t, in_=fj[cc * P : (cc + 1) * P, P:FJA])
                fja_sb.append(t)
            for cc in range(KC):
                t = singles.tile([P, JJ - FJA], mmdt, tag=f"fjb_{cc}")
                nc.sync.dma_start(out=t, in_=fj[cc * P : (cc + 1) * P, FJA:JJ])
                fjb_sb.append(t)

            def fj_cols(cc, jt):
                if jt == 0:
                    return fj0_sb[cc]
                if jt < 4:
                    return fja_sb[cc][:, (jt - 1) * P : jt * P]
                return fjb_sb[cc][:, (jt - 4) * P : (jt - 3) * P]

            for jt in range(NJT):
                exp_t = exp_pool.tile([P, HW], FP32, tag="exp")
                m = stats.tile([P, NQ], FP32, tag="m")  # m_q
                nm = stats.tile([P, NQ], FP32, tag="nm")  # -m_q
                sums = stats.tile([P, NQ], FP32, tag="sums")
                for q in range(NQ):
                    ps = ps_pool.tile([P, QW], FP32, tag="ps")
                    for sub in range(QW // MMN):
                        ioff = q * QW + sub * MMN
                        half, loff = divmod(ioff, HB)
                        for cc in range(KC):
                            nc.tensor.matmul(
                                ps[:, bass.ts(sub, MMN)],
                                lhsT=fj_cols(cc, jt),
                                rhs=fi_sb[half * KC + cc][:, loff : loff + MMN],
                                start=(cc == 0),
                                stop=(cc == KC - 1),
                            )
                    nc.vector.reduce_max(
                        out=m[:, q : q + 1], in_=ps, axis=mybir.AxisListType.X
                    )
                    nc.vector.tensor_scalar_mul(
                        nm[:, q : q + 1], m[:, q : q + 1], -1.0
                    )
                    nc.scalar.activation(
                        out=exp_t[:, q * QW : (q + 1) * QW],
                        in_=ps,
                        func=mybir.ActivationFunctionType.Exp,
                        bias=nm[:, q : q + 1],
                        scale=1.0,
                        accum_out=sums[:, q : q + 1],
                    )
                # merge: M = max_q m_q; e_q = exp(m_q - M); S = sum sums_q*e_q
                nM = stats.tile([P, 1], FP32, tag="nM")  # -M
                nc.vector.reduce_max(
                    out=nM, in_=m, axis=mybir.AxisListType.X, negate=True
                )
                eq = stats.tile([P, NQ], FP32, tag="eq")
                nc.scalar.activation(
                    out=eq,
                    in_=m,
                    func=mybir.ActivationFunctionType.Exp,
                    bias=nM,
                    scale=1.0,
                )
                w = stats.tile([P, NQ], FP32, tag="w")
                nc.vector.tensor_mul(w, sums, eq)
                S = stats.tile([P, 1], FP32, tag="S")
                nc.vector.reduce_sum(out=S, in_=w, axis=mybir.AxisListType.X)
                rs = stats.tile([P, 1], FP32, tag="rs")
                nc.vector.reciprocal(out=rs, in_=S)
                r = stats.tile([P, NQ], FP32, tag="r")
                nc.vector.tensor_scalar_mul(r, eq, rs)
                # normalize + stream out: DVE takes q3 (it owns r, finishes
                # first), Pool then q0..q2 -- DMAs ladder out in that order.
                nc.vector.tensor_scalar_mul(
                    exp_t[:, 3 * QW : 4 * QW], exp_t[:, 3 * QW : 4 * QW], r[:, 3:4]
                )
                nc.sync.dma_start(
                    out=out[jt * P : (jt + 1) * P, 3 * QW : 4 * QW],
                    in_=exp_t[:, 3 * QW : 4 * QW],
                )
                for q in range(3):
                    nc.gpsimd.tensor_scalar_mul(
                        exp_t[:, q * QW : (q + 1) * QW],
                        exp_t[:, q * QW : (q + 1) * QW],
                        r[:, q : q + 1],
                    )
                    nc.sync.dma_start(
                        out=out[jt * P : (jt + 1) * P, q * QW : (q + 1) * QW],
                        in_=exp_t[:, q * QW : (q + 1) * QW],
                    )
    return nc


_NC = None


def _get_nc():
    global _NC
    if _NC is None:
        _NC = _build_bass()
    return _NC


def _run(feat1, feat2, trace=False):
    f1 = np.asarray(feat1, dtype=np.float32).reshape(B, C, HW)
    f2 = np.asarray(feat2, dtype=np.float32).reshape(B, C, HW)
    in_maps = []
    for d in range(N_CORES):
        bb, jh = d // 2, d % 2
        in_maps.append(
            {
                "fj": np.ascontiguousarray(f2[bb][:, jh * JJ : (jh + 1) * JJ]),
                "fi": np.ascontiguousarray(f1[bb]),
            }
        )
    res = run_bass_kernel_spmd(_get_nc(), in_maps, list(range(N_CORES)), trace=trace)
    out = np.empty((B, HW, HW), np.float32)
    for d in range(N_CORES):
        bb, jh = d // 2, d % 2
        # device tile is (j_local, i); transpose during unshard
        out[bb][:, jh * JJ : (jh + 1) * JJ] = res.results[d]["out"].T
    return out.reshape(B, HW, H, W), res


def kernel(feat1, feat2):
    out, _ = _run(feat1, feat2)
    return out


# revision 15
# speedup vs baseline: 1.2377x; 1.0101x over previous
"""Correlation layer + softmax(axis=i) Trainium2 kernel.

corr[b,i,j] = sum_c f1[b,c,i] * f2[b,c,j]   (b=4, c=256, i,j in hw=4096)
out = softmax(corr, axis=i) reshaped to (4, 4096, 64, 64)

Sharding: 8 cores = 4 batches x 2 j-halves. Softmax reduces over i, which is
fully local per core when corr is computed transposed (j on partitions, i on
the free axis).

Per core (2048 j x 4096 i), for each of 16 j-tiles (128 j), a chunked
(flash-style) softmax over four 1024-wide i-quarters:
  1. corrT quarter (128 j, 1024 i) = f2_cols.T @ f1 via 4 fp32r matmuls into
     a 2-bank PSUM tile (pool bufs=4 -> all 8 banks, quarters pipeline
     independently),
  2. per-quarter row max m_q (DVE, negated) then exp(corr - m_q) straight off
     PSUM in ONE activation per quarter (amortizes PSUM-access + accumulator
     overhead), per-quarter row sums accumulate on the fly. Using the LOCAL
     quarter max keeps every exp in [0,1] -- overflow-safe for any input and,
     unlike a global row max, never serializes PSUM recycling across quarters.
  3. merge: M = max_q m_q, e_q = exp(m_q - M), S = sum_q sums_q*e_q,
     r_q = e_q / S  (tiny [128,4] ops on DVE + one small ACT exp),
  4. normalize quarter q by r_q -- Pool takes q0..q2, DVE takes q3 (balances
     both engines under the DMA roofline) -- and DMA each quarter out as soon
     as it is scaled.
Input DMAs are chunked in need-order (j-tile-0 cols, f1 halves, remaining f2
cols) so matmuls start ~2us in; output DMA saturates from ~20us on. The
device output is (2048 j, 4096 i) per core; the host transposes during
unsharding (the gather has to copy these bytes anyway).

This walrus build allows only ONE sync wait per instruction. Tile freely
emits several, so kernel.py patches two spots in the Tile pipeline:
  - a post-scheduling pass splits every multi-wait instruction into
    single-wait same-engine Drain carriers ahead of it,
  - the kernel-tail drain (one wait per outstanding semaphore) is split the
    same way.
"""

import sys

import numpy as np

sys.path.insert(0, "/opt/trn_rl_repo")

import concourse.bass as bass
import concourse.mybir as mybir
import concourse.tile as tile
from concourse.bass_utils import run_bass_kernel_spmd

B, C, H, W = 4, 256, 64, 64
HW = H * W  # 4096
JJ = HW // 2  # j columns per core
N_CORES = 8
P = 128
KC = C // P  # 2 contraction chunks
NJT = JJ // P  # 16 j-tiles per core
HB = 2048  # input-load half width
QW = 1024  # softmax quarter width = 2 PSUM banks
NQ = HW // QW  # 4
MMN = 512  # matmul moving width (one PSUM bank)

FP32 = mybir.dt.float32

_split_counter = [0]


def _split_multiwaits(ordered):
    """Walrus (this build) rejects instructions with >1 sync wait. Hoist the
    extra waits onto single-wait Drain instructions on the same engine queue
    immediately before the offender (queues are in-order)."""
    for bb, insts in ordered.items():
        out = []
        changed = False
        for inst in insts:
            si = getattr(inst, "sync_info", None)
            waits = list(si.on_wait) if (si is not None and si.on_wait) else []
            if len(waits) > 1:
                changed = True
                for w in waits[:-1]:
                    _split_counter[0] += 1
                    d = mybir.InstDrain(
                        name=f"I-wsplit-{_split_counter[0]}",
                        ins=[],
                        outs=[],
                        engine=inst.engine,
                    )
                    d.sync_info = mybir.SyncInfo(on_wait=[w], on_update=[])
                    out.append(d)
                si.on_wait = waits[-1:]
            out.append(inst)
        if changed:
            ordered[bb] = out
    return ordered


_orig_postorder = tile.postorder_instruction_blocks


def _patched_postorder(ordered, start_bb_name, postordered):
    _split_multiwaits(ordered)
    return _orig_postorder(ordered, start_bb_name, postordered)


tile.postorder_instruction_blocks = _patched_postorder


def _patched_drain_and_barrier(self, tick_clock, wait_clock):
    """Same single-wait discipline for the kernel-tail drain."""
    from concourse.vector_clock import ScopedClock

    drain_inst = self.nc.sync.drain()
    wait_clock.add_sem_waits(
        drain_inst.ins, ScopedClock({None: tick_clock.global_clock})
    )
    si = drain_inst.ins.sync_info
    waits = list(si.on_wait or []) if si is not None else []
    if len(waits) > 1:
        si.on_wait = waits[:1]
        for w in waits[1:]:
            d2 = self.nc.sync.drain()
            si2 = d2.ins.sync_info
            if si2 is None:
                d2.ins.sync_info = mybir.SyncInfo(on_wait=[w], on_update=[])
            else:
                si2.on_wait = [w]
    self.nc.all_engine_barrier()
    assert self.sems is not None
    popped = self.nc._tile_sem_poison_stack.pop()
    assert popped is self._sem_poison
    self.nc.clear_and_free_semaphores(list(self.sems.allocated().values()))
    self.nc.all_engine_barrier()


tile.TileContext._drain_and_barrier = _patched_drain_and_barrier


def _build_bass():
    nc = bass.Bass()
    mmdt = mybir.dt.float32r
    fj = nc.declare_dram_parameter("fj", [C, JJ], mmdt, isOutput=False)
    fi = nc.declare_dram_parameter("fi", [C, HW], mmdt, isOutput=False)
    out = nc.declare_dram_parameter("out", [JJ, HW], FP32, isOutput=True)

    with tile.TileContext(nc) as tc:
        with (
            tc.tile_pool(name="singles", bufs=1) as singles,
            tc.tile_pool(name="exp", bufs=4) as exp_pool,
            tc.tile_pool(name="stats", bufs=24) as stats,
            tc.tile_pool(name="ps", bufs=4, space="PSUM") as ps_pool,
        ):
            # Input DMAs in need-order. Separate tiles per chunk so a matmul
            # only waits on the chunk it actually reads.
            fj0_sb, fja_sb, fjb_sb, fi_sb = [], [], [], []
            # fj0 on the SWDGE (gpsimd) queue so it issues in parallel with
            # the SP queue's fi stream.
            for cc in range(KC):
                t = singles.tile([P, P], mmdt, tag=f"fj0_{cc}")
                nc.gpsimd.dma_start(out=t, in_=fj[cc * P : (cc + 1) * P, 0:P])
                fj0_sb.append(t)
            for half in range(2):
                for cc in range(KC):
                    t = singles.tile([P, HB], mmdt, tag=f"fi_{half}_{cc}")
                    nc.sync.dma_start(
                        out=t,
                        in_=fi[cc * P : (cc + 1) * P, half * HB : (half + 1) * HB],
                    )
                    fi_sb.append(t)  # index = half*KC + cc
            # all of fi lands first; the fj tail then streams in during tile
            # 0's softmax-merge latency instead of in front of it.
            FJA = 4 * P
            for cc in range(KC):
                t = singles.tile([P, FJA - P], mmdt, tag=f"fja_{cc}")
                nc.sync.dma_start(out=t, in_=fj[cc * P : (cc + 1) * P, P:FJA])
                fja_sb.append(t)
            for cc in range(KC):
                t = singles.tile([P, JJ - FJA], mmdt, tag=f"fjb_{cc}")
                nc.sync.dma_start(out=t, in_=fj[cc * P : (cc + 1) * P, FJA:JJ])
                fjb_sb.append(t)

            def fj_cols(cc, jt):
                if jt == 0:
                    return fj0_sb[cc]
                if jt < 4:
                    return fja_sb[cc][:, (jt - 1) * P : jt * P]
                return fjb_sb[cc][:, (jt - 4) * P : (jt - 3) * P]

            for jt in range(NJT):
                exp_t = exp_pool.tile([P, HW], FP32, tag="exp")
                nm = stats.tile([P, NQ], FP32, tag="nm")  # -m_q
                sums = stats.tile([P, NQ], FP32, tag="sums")
                for q in range(NQ):
                    ps = ps_pool.tile([P, QW], FP32, tag="ps")
                    for sub in range(QW // MMN):
                        ioff = q * QW + sub * MMN
                        half, loff = divmod(ioff, HB)
                        for cc in range(KC):
                            nc.tensor.matmul(
                                ps[:, bass.ts(sub, MMN)],
                                lhsT=fj_cols(cc, jt),
                                rhs=fi_sb[half * KC + cc][:, loff : loff + MMN],
                                start=(cc == 0),
                                stop=(cc == KC - 1),
                            )
                    nc.vector.reduce_max(
                        out=nm[:, q : q + 1],
                        in_=ps,
                        axis=mybir.AxisListType.X,
                        negate=True,
                    )
                    nc.scalar.activation(
                        out=exp_t[:, q * QW : (q + 1) * QW],
                        in_=ps,
                        func=mybir.ActivationFunctionType.Exp,
                        bias=nm[:, q : q + 1],
                        scale=1.0,
                        accum_out=sums[:, q : q + 1],
                    )
                # merge: M = max_q m_q; e_q = exp(m_q - M); S = sum sums_q*e_q
                nM = stats.tile([P, 1], FP32, tag="nM")  # -M
                nc.vector.tensor_reduce(
                    out=nM, in_=nm, axis=mybir.AxisListType.X, op=mybir.AluOpType.min
                )
                eq = stats.tile([P, NQ], FP32, tag="eq")
                nc.scalar.activation(
                    out=eq,
                    in_=nm,
                    func=mybir.ActivationFunctionType.Exp,
                    bias=nM,
                    scale=-1.0,
                )
                w = stats.tile([P, NQ], FP32, tag="w")
                nc.vector.tensor_mul(w, sums, eq)
                S = stats.tile([P, 1], FP32, tag="S")
                nc.vector.reduce_sum(out=S, in_=w, axis=mybir.AxisListType.X)
                rs = stats.tile([P, 1], FP32, tag="rs")
                nc.vector.reciprocal(out=rs, in_=S)
                r = stats.tile([P, NQ], FP32, tag="r")
                nc.vector.tensor_scalar_mul(r, eq, rs)
                # normalize + stream out: DVE takes q3 (it owns r, finishes
                # first), Pool then q0..q2 -- DMAs ladder out in that order.
                nc.vector.tensor_scalar_mul(
                    exp_t[:, 3 * QW : 4 * QW], exp_t[:, 3 * QW : 4 * QW], r[:, 3:4]
                )
                nc.sync.dma_start(
                    out=out[jt * P : (jt + 1) * P, 3 * QW : 4 * QW],
                    in_=exp_t[:, 3 * QW : 4 * QW],
                )
                for q in range(3):
                    nc.gpsimd.tensor_scalar_mul(
                        exp_t[:, q * QW : (q + 1) * QW],
                        exp_t[:, q * QW : (q + 1) * QW],
                        r[:, q : q + 1],
                    )
                    nc.sync.dma_start(
                        out=out[jt * P : (jt + 1) * P, q * QW : (q + 1) * QW],
                        in_=exp_t[:, q * QW : (q + 1) * QW],
                    )
    return nc


_NC = None


def _get_nc():
    global _NC
    if _NC is None:
        _NC = _build_bass()
    return _NC


def _run(feat1, feat2, trace=False):
    f1 = np.asarray(feat1, dtype=np.float32).reshape(B, C, HW)
    f2 = np.asarray(feat2, dtype=np.float32).reshape(B, C, HW)
    in_maps = []
    for d in range(N_CORES):
        bb, jh = d // 2, d % 2
        in_maps.append(
            {
                "fj": np.ascontiguousarray(f2[bb][:, jh * JJ : (jh + 1) * JJ]),
                "fi": np.ascontiguousarray(f1[bb]),
            }
        )
    res = run_bass_kernel_spmd(_get_nc(), in_maps, list(range(N_CORES)), trace=trace)
    out = np.empty((B, HW, HW), np.float32)
    for d in range(N_CORES):
        bb, jh = d // 2, d % 2
        # device tile is (j_local, i); transpose during unshard
        out[bb][:, jh * JJ : (jh + 1) * JJ] = res.results[d]["out"].T
    return out.reshape(B, HW, H, W), res


def kernel(feat1, feat2):
    out, _ = _run(feat1, feat2)
    return out
